# revision 37
# baseline (speedup 1.0000x reference)
"""Pairwise Euclidean distance kernel for Trainium2 (8 NeuronCores, SPMD).

Problem: mapping [8192, 256] f32 -> out [8192, 8192] f32 where
out[i, j] = ||mapping[i] - mapping[j]||_2, via the GEMM identity
d2 = ||x_i||^2 + ||x_j||^2 - 2 <x_i, x_j>.

Sharding: row-block of the output per core. Core c computes rows
[c*1024, (c+1)*1024) against all 8192 columns. To keep one SPMD program
with static addressing, each core's inputs are rotated by c*1024 (rows of
the natural layout / columns of the transposed layout); the host un-rotates
each core's output columns afterwards.

Per-core on-device pipeline:
  - inputs: mt [256, 8192] f16 (x^T, rotated), nat [8192, 256] f16 (x,
    rotated), eye [128, 128] f16 (transpose identity)
  - sq_j = sum_k x~[j,k]^2 in f32 on DVE (square + 3D reduce), where x~ is
    the f16-rounded input; using the same rounded values for the gram and
    for sq makes the diagonal cancel to ~1e-4.
  - The -0.5*sq_j row is split hi/lo into two f16 rows (exact to ~2^-22) and
    folded into the PSUM accumulation as a K=2 rank-1 matmul with an all-ones
    stationary operand: psum = gram - 0.5*sq_j.
  - ACT computes sqrt(-2*psum + sq_i) with per-partition bias sq_i, reading
    PSUM directly. d2 can only go negative (fp rounding) where the true
    distance is 0, i.e. the i==j block, so a [128,128] tensor_scalar_min
    clamp (psum <= 0.5*sq_i) before ACT protects exactly that block.
  - matmul dtype f16: PE multiplies f16 exactly into f32 PSUM; the only
    error vs the f32 reference is the input rounding (~2e-4 relative).
"""

import sys

try:
    import concourse.bass as _probe  # noqa: F401
except ImportError:
    sys.path.insert(0, "/opt/trn_rl_repo")

import numpy as np

import concourse.bacc as bacc
import concourse.mybir as mybir
from concourse import tile
from concourse.bass_utils import run_bass_kernel_spmd

N = 8192          # number of points
D = 256           # feature dim
NCORES = 8
RPC = N // NCORES  # 1024 rows per core
RT = RPC // 128    # 8 row-tiles per core
JCHUNK = 2048      # output chunk width (4 PSUM banks)
NJC = N // JCHUNK  # 4 chunks
NSUB = JCHUNK // 512  # 4 matmul sub-tiles per chunk
NGRP = 8           # sq reduction groups (8 tiles of 128 rows each)

F16 = mybir.dt.float16
F32 = mybir.dt.float32


def _build_nc(repeats=1, loop_n=None, stage_bufs=4, work_bufs=2):
    nc = bacc.Bacc(None, target_bir_lowering=False)
    mt_d = nc.dram_tensor("mt", [D, N], F16, kind="ExternalInput")
    nat_d = nc.dram_tensor("nat", [N, D], F16, kind="ExternalInput")
    eye_d = nc.dram_tensor("eye", [128, 128], F32, kind="ExternalInput")
    out_d = nc.dram_tensor("out", [RPC, N], F32, kind="ExternalOutput")

    with tile.TileContext(nc) as tc:
        with (
            tc.tile_pool(name="big", bufs=1) as big,
            tc.tile_pool(name="work", bufs=work_bufs) as work,
            tc.tile_pool(name="stage", bufs=stage_bufs) as stage_pool,
            tc.tile_pool(name="ps", bufs=2, space="PSUM") as psum,
        ):
            if loop_n is not None:
                with tc.For_i(0, loop_n, 1):
                    _emit_body(nc, tc, big, work, stage_pool, psum,
                               mt_d, nat_d, eye_d, out_d)
            else:
                for _rep in range(repeats):
                    _emit_body(nc, tc, big, work, stage_pool, psum,
                               mt_d, nat_d, eye_d, out_d)

    nc.compile()
    return nc


def _emit_body(nc, tc, big, work, stage_pool, psum, mt_d, nat_d, eye_d, out_d):
    # --- persistent SBUF tensors; mt loaded in 2048-column chunks so the
    # first main-loop chunk only depends on the first slice ---
    mt0 = big.tile([128, N], F16, tag="mt0")
    mt1 = big.tile([128, N], F16, tag="mt1")
    eye = big.tile([128, 128], F32, tag="eye")
    ones2 = big.tile([2, 128], F16, tag="ones2")
    # per-pair sq tensors: a single shared tile would create false
    # WAR/RAW couplings (later pairs write other slices while every chunk
    # reads its bias / rank-1 row), serializing the pipeline
    sqp = []
    sqf = []
    for _p in range(NJC):
        sqp_t = big.tile([128, 16], F32, tag=f"sqp{_p}")
        sqp.append(sqp_t)
        sqf_t = big.tile([2, JCHUNK], F16, tag=f"sqf{_p}")
        sqf.append(sqf_t)

    nat_g = nat_d.rearrange("(g t p) d -> g p t d", g=NGRP, p=128)

    # input DMAs are staggered: the DMA pool drains a serial queue, so bulk
    # loads issued too early would delay the small latency-critical flatten
    # DMAs behind megabytes of queued transfers
    gts = {}
    for g in range(NGRP):
        gt_slot = big.tile([128, 8, 256], F16, tag=f"natg{g}")
        gts[g] = gt_slot
    nc.sync.dma_start(gts[0][:], nat_g[0])
    nc.sync.dma_start(gts[1][:], nat_g[1])
    nc.sync.dma_start(mt0[:, 0:JCHUNK], mt_d[0:128, 0:JCHUNK])
    nc.sync.dma_start(mt1[:, 0:JCHUNK], mt_d[128:256, 0:JCHUNK])
    nc.sync.dma_start(eye[:], eye_d[:])

    def emit_loads(stage):
        # stage 0 at post-pair-0, stages 1..2 late in jc 0..1
        g0 = 2 + 2 * stage
        nc.sync.dma_start(gts[g0][:], nat_g[g0])
        nc.sync.dma_start(gts[g0 + 1][:], nat_g[g0 + 1])
        j1 = (stage + 1) * JCHUNK
        nc.sync.dma_start(mt0[:, j1:j1 + JCHUNK], mt_d[0:128, j1:j1 + JCHUNK])
        nc.sync.dma_start(mt1[:, j1:j1 + JCHUNK], mt_d[128:256, j1:j1 + JCHUNK])

    EARLY_LOADS = True

    def emit_sq_reduce(pair):
        # sq for j in [pair*2048, (pair+1)*2048): nat groups 2p, 2p+1 ->
        # sq_tiles[:, 16p:16p+16] -> -0.5 hi/lo f16 slices (all DVE work).
        # pair 0 is on the critical path: fused square+reduce (one pass per
        # 256-wide tile) halves its DVE latency vs mul-then-reduce.
        for g in (2 * pair, 2 * pair + 1):
            gt = gts[g]
            gl = g - 2 * pair
            # square on ACT (plain Square, no accum - the fused/accum DVE and
            # ACT variants crash this hardware), reduce on DVE: splits the sq
            # work across both engines and keeps the DVE queue shallow
            msq = work.tile([128, 8, 256], F32, tag="msq")
            nc.scalar.activation(msq[:], gt[:],
                                 mybir.ActivationFunctionType.Square)
            nc.vector.reduce_sum(
                sqp[pair][:, gl * 8:(gl + 1) * 8].unsqueeze(2),
                msq[:],
                axis=mybir.AxisListType.X,
            )
        sl = sqp[pair][:, 0:16]
        mh32 = work.tile([128, 16], F32, tag=f"mh32_{pair}")
        nc.vector.tensor_scalar_mul(mh32[:], sl, -0.5)
        hi16 = work.tile([128, 16], F16, tag=f"hi16_{pair}")
        nc.vector.tensor_copy(hi16[:], mh32[:])
        hi32 = work.tile([128, 16], F32, tag=f"hi32_{pair}")
        nc.vector.tensor_copy(hi32[:], hi16[:])
        lo32 = work.tile([128, 16], F32, tag=f"lo32_{pair}")
        nc.vector.tensor_sub(lo32[:], mh32[:], hi32[:])
        return mh32, lo32

    def emit_sq_flatten(pair, mh32, lo32):
        # transpose [128, 16] -> [16, 128] on PE, flatten into sq_flat; kept
        # separate so the in-order PE only meets these after the DVE chain
        # has had time to produce mh32/lo32
        for row, src in ((0, mh32), (1, lo32)):
            pt = psum.tile([16, 128], F32, tag="ps")
            nc.tensor.transpose(pt[:], src[:], eye[:])
            st = work.tile([16, 128], F16, tag="sqT")
            nc.vector.tensor_copy(st[:], pt[:])
            nc.sync.dma_start(
                sqf[pair][row:row + 1, :].rearrange("o (t i) -> o t i", t=16),
                st[:],
            )

    # pair-0 sq chain first (the first rank-1 matmul blocks the in-order PE
    # stream until sq_flat[:, 0:2048] lands); high priority so the scheduler
    # does not interleave later pairs' DVE work into this chain
    nc.vector.memset(ones2[:], 1.0)
    with tc.high_priority():
        emit_sq_flatten(0, *emit_sq_reduce(0))
    emit_loads(0)
    emit_loads(1)
    emit_loads(2)

    # --- main loop: chunk-outer so chunk 0 starts as soon as its sq slice
    # and mt slice are resident ---
    for jc in range(NJC):
        nxt = None
        def emit_kmms(ps, r):
            lhs0 = mt0[:, r * 128:(r + 1) * 128]
            lhs1 = mt1[:, r * 128:(r + 1) * 128]
            for s in range(NSUB):
                j0 = jc * JCHUNK + s * 512
                o = ps[:, s * 512:(s + 1) * 512]
                nc.tensor.matmul(o, lhs0, mt0[:, j0:j0 + 512],
                                 start=True, stop=False)
                nc.tensor.matmul(o, lhs1, mt1[:, j0:j0 + 512],
                                 start=False, stop=False)

        def emit_rank1(ps):
            for s in range(NSUB):
                o = ps[:, s * 512:(s + 1) * 512]
                nc.tensor.matmul(o, ones2[:], sqf[jc][:, s * 512:(s + 1) * 512],
                                 start=False, stop=True)

        def emit_tail(ps, r):
            out_t = stage_pool.tile([128, JCHUNK], F32, tag="stage")
            bias = sqp[0][:, r:r + 1]
            if jc == 0:
                # d2 can only go negative (fp rounding) in the i==j block,
                # and ACT Sqrt requires inputs >= 0: compute that 128-wide
                # slice as sqrt(relu(d2)) and the flanks as plain sqrt, all
                # on ACT (a DVE clamp here would stall chunk retirement
                # behind the in-order DVE's sq backlog)
                c0, c1 = r * 128, (r + 1) * 128
                relu_t = work.tile([128, 128], F32, tag="relu")
                nc.scalar.activation(
                    relu_t[:], ps[:, c0:c1],
                    mybir.ActivationFunctionType.Relu,
                    bias=bias, scale=-2.0,
                )
                if r > 0:
                    nc.scalar.activation(
                        out_t[:, 0:c0], ps[:, 0:c0],
                        mybir.ActivationFunctionType.Sqrt,
                        bias=bias, scale=-2.0,
                    )
                nc.scalar.activation(
                    out_t[:, c0:c1], relu_t[:],
                    mybir.ActivationFunctionType.Sqrt,
                )
                nc.scalar.activation(
                    out_t[:, c1:JCHUNK], ps[:, c1:JCHUNK],
                    mybir.ActivationFunctionType.Sqrt,
                    bias=bias, scale=-2.0,
                )
            else:
                nc.scalar.activation(
                    out_t[:], ps[:],
                    mybir.ActivationFunctionType.Sqrt,
                    bias=bias, scale=-2.0,
                )
            nc.sync.dma_start(
                out_d[r * 128:(r + 1) * 128,
                      jc * JCHUNK:(jc + 1) * JCHUNK],
                out_t[:],
            )

        if jc == 0:
            # fill both psum slots with sq-independent k-matmuls first so
            # the in-order PE has runway while the sq chain completes
            ps0 = psum.tile([128, JCHUNK], F32, tag="ps")
            emit_kmms(ps0, 0)
            ps1 = psum.tile([128, JCHUNK], F32, tag="ps")
            emit_kmms(ps1, 1)
            emit_rank1(ps0)
            emit_tail(ps0, 0)
            emit_rank1(ps1)
            emit_tail(ps1, 1)
            start_r = 2
        else:
            start_r = 0
        for r in range(start_r, RT):
            if r == start_r and jc + 1 < NJC:
                nxt = emit_sq_reduce(jc + 1)
            if r == start_r + 1 and nxt is not None:
                emit_sq_flatten(jc + 1, *nxt)

            ps = psum.tile([128, JCHUNK], F32, tag="ps")
            emit_kmms(ps, r)
            emit_rank1(ps)
            emit_tail(ps, r)


_NC_CACHE = None


def _get_nc():
    global _NC_CACHE
    if _NC_CACHE is None:
        _NC_CACHE = _build_nc()
    return _NC_CACHE


def kernel(mapping: np.ndarray, **_kwargs) -> np.ndarray:
    mapping = np.asarray(mapping, dtype=np.float32)
    assert mapping.shape == (N, D)
    xh = mapping.astype(np.float16)
    eye = np.eye(128, dtype=np.float32)

    in_maps = []
    for c in range(NCORES):
        natc = np.ascontiguousarray(np.roll(xh, -c * RPC, axis=0))
        mtc = np.ascontiguousarray(natc.T)
        in_maps.append({"mt": mtc, "nat": natc, "eye": eye})

    nc = _get_nc()
    res = run_bass_kernel_spmd(nc, in_maps, core_ids=list(range(NCORES)))

    out = np.empty((N, N), dtype=np.float32)
    for c in range(NCORES):
        out[c * RPC:(c + 1) * RPC] = np.roll(res.results[c]["out"], c * RPC, axis=1)
    return out


if __name__ == "__main__":
    rng = np.random.default_rng(0)
    x = rng.standard_normal((N, D)).astype(np.float32)
    o = kernel(mapping=x)
    print("out", o.shape, o.dtype, "sample", o[0, :4], "diag", np.abs(np.diag(o)).max())
